# revision 13
# baseline (speedup 1.0000x reference)
"""MAD predictor (retrieval_knn) — Trainium2 Bass/Tile kernel on 8 NeuronCores.

kernel(**inputs) takes the FULL inputs and returns the FULL (4096,) f32 output.
Sharding: batch edges split 512/core across 8 cores; embeds replicated (bf16).
Per-edge gathers of *inputs* (adjacency rows/cols, field rows, x.g dots) are
host-staged as part of sharding; everything depending on *computed* k-NN
samples runs on device.

Per core, per head h and build (src->dst, dst->src), per 128-edge row tile:
  S[b,n] = 2*x_b.e_n - |e_n|^2   PE matmul in 5 groups of 4x512 cols + K=1
                                 bias matmul (negy2 host-precomputed, cols
                                 padded to 10240 with -30000)
  ACT copies each [128,2048] PSUM group to SBUF bf16.
  DVE fold cascade (tensor_tensor max, 2x bf16): 10240->5120->2560->1280;
  block max8 + match_replace merge on the 1280-wide fold gives self (m1[0])
  and the 8 nearest neighbors nv; find_index8 on the 1280-wide fold + 3
  gather/compare resolve levels (gpsimd indirect_copy) recover column ids.
  One batched indirect DMA gathers the 8 neighbor embedding rows; EG_k =
  e_s.g_b via gpsimd bcast-multiply + DVE grouped reduce; adjacency bits via
  indirect DMA from host-staged adj rows/cols (uint8).
  d2_k = S_self - S_k; per-rt batched epilogue: dist=sqrt(d2), w=exp(1-dist),
  logit = xg - EG + u*(2*adj-1); softmin_h = sum w*logit/(8+sum w);
  out = sigmoid(mean_h softmin_h).
"""

import sys
from contextlib import ExitStack

for _p in ('/opt/trn_rl_repo', '/root/.axon_site/_ro/trn_rl_repo'):
    if _p not in sys.path:
        sys.path.append(_p)

import numpy as np
import ml_dtypes

import concourse.bass as bass
import concourse.bacc as bacc
import concourse.mybir as mybir
from concourse.tile import TileContext
from concourse.bass_utils import run_bass_kernel_spmd

BF16 = mybir.dt.bfloat16
F32 = mybir.dt.float32
U32 = mybir.dt.uint32
U16 = mybir.dt.uint16
U8 = mybir.dt.uint8
P = 128
NEG_BIG = -3.0e38
bf = ml_dtypes.bfloat16

# problem constants (hardcoded per contract)
H, N, D = 4, 10000, 128
B, NCORES = 4096, 8
NB = B // NCORES          # 512 edges per core
RT = NB // P              # 4 row-tiles of 128 edges
NSENT = 8
NPAD = 10240              # 20 chunks of 512 exactly
ALIST = 32                # max adjacency-ones per edge side (P(>32) ~ 0)
GROUPS, GW = 5, 2048      # 5 PSUM groups of 4x512
NU = 2 * H                # (h, bu) units per rt


def build_kernel(u):
    nc = bacc.Bacc("TRN2", target_bir_lowering=False, debug=False,
                   enable_asserts=True, num_devices=NCORES)

    eT = nc.declare_dram_parameter("eT", [H, D, NPAD], BF16, isOutput=False)
    negy2 = nc.declare_dram_parameter("negy2", [H, 1, NPAD], BF16, isOutput=False)
    xT2 = nc.declare_dram_parameter("xT2", [H, 2, D, NB], BF16, isOutput=False)
    grows = nc.declare_dram_parameter("grows", [H, 2, NB, D], BF16, isOutput=False)
    xg_d = nc.declare_dram_parameter("xg", [RT, P, NU], F32, isOutput=False)
    emb = {h: nc.declare_dram_parameter(f"emb_{h}", [N, D], BF16, isOutput=False)
           for h in range(H)}
    # per edge-side adjacency: the <=ALIST column ids n with adj-label 1
    alist_d = nc.declare_dram_parameter("alist", [2, RT, P, ALIST], U16,
                                        isOutput=False)
    out_p = nc.declare_dram_parameter("out", [NB, 1], F32, isOutput=True)

    with TileContext(nc) as tc, ExitStack() as ctx:
        pconst = ctx.enter_context(tc.tile_pool(name="const", bufs=1))
        peT = ctx.enter_context(tc.tile_pool(name="peT", bufs=2))
        pS = ctx.enter_context(tc.tile_pool(name="pS", bufs=3))
        pfold = ctx.enter_context(tc.tile_pool(name="pfold", bufs=2))
        psmall = ctx.enter_context(tc.tile_pool(name="small", bufs=3))
        pacc = ctx.enter_context(tc.tile_pool(name="acc", bufs=1))
        pepi = ctx.enter_context(tc.tile_pool(name="epi", bufs=1))
        pny2 = ctx.enter_context(tc.tile_pool(name="ny2", bufs=1))
        pidx = ctx.enter_context(tc.tile_pool(name="idx", bufs=8))
        ppsum = ctx.enter_context(tc.tile_pool(name="psum", bufs=2, space="PSUM"))

        ones_row = pconst.tile([1, P], BF16)
        nc.vector.memset(ones_row[:], 1.0)
        alist_t = {}
        for bu in range(2):
            for rt in range(RT):
                alist_t[(bu, rt)] = pconst.tile([P, ALIST], U16,
                                                tag=f"al{bu}_{rt}",
                                                name=f"al{bu}_{rt}")
                nc.sync.dma_start(out=alist_t[(bu, rt)][:], in_=alist_d[bu, rt])

        # batched-epilogue accumulators, one set per row tile (flat layout)
        d2all = {rt: pacc.tile([P, NU * 8], F32, tag=f"d2all{rt}",
                               name=f"d2all{rt}") for rt in range(RT)}
        egall = {rt: pacc.tile([P, NU * 8], F32, tag=f"egall{rt}",
                               name=f"egall{rt}") for rt in range(RT)}
        a8all = {rt: pacc.tile([P, NU * 8], F32, tag=f"a8all{rt}",
                               name=f"a8all{rt}") for rt in range(RT)}
        xgall = {rt: pacc.tile([P, NU], F32, tag=f"xgall{rt}",
                               name=f"xgall{rt}") for rt in range(RT)}
        for rt in range(RT):
            nc.sync.dma_start(out=xgall[rt][:], in_=xg_d[rt])

        # PE warmup: sustained matmuls un-throttle the HAM clock gate.
        wsrc = pconst.tile([P, 512], BF16)
        nc.vector.memset(wsrc[:], 0.001)
        wps = ppsum.tile([P, GW], F32, tag="psS")
        for i in range(16):
            nc.tensor.matmul(wps[:, :512], lhsT=wsrc[:, :P], rhs=wsrc[:],
                             start=True, stop=True)
        wout = pconst.tile([1, 1], F32)
        nc.vector.tensor_copy(wout[:], wps[:1, :1])

        pending_eg = []

        def flush_eg():
            while pending_eg:
                p3p, dstp = pending_eg.pop(0)
                nc.vector.tensor_reduce(dstp, p3p, axis=mybir.AxisListType.X,
                                        op=mybir.AluOpType.add)

        for h in range(H):
            eTh = peT.tile([D, NPAD], BF16, tag="eTh")
            nc.sync.dma_start(out=eTh[:], in_=eT[h])
            ny2h = pny2.tile([1, NPAD], BF16, tag="ny2h")
            nc.sync.dma_start(out=ny2h[:], in_=negy2[h])

            for bu in range(2):
                iu = h * 2 + bu
                xTs = psmall.tile([D, NB], BF16, tag="xTs")
                nc.sync.dma_start(out=xTs[:], in_=xT2[h, bu])

                for rt in range(RT):
                    S_sb = pS.tile([P, NPAD], BF16, tag="S_sb")
                    for g in range(GROUPS):
                        ps = ppsum.tile([P, GW], F32, tag="psS")
                        for q in range(4):
                            c0 = g * GW + q * 512
                            nc.tensor.matmul(ps[:, q * 512:(q + 1) * 512],
                                             lhsT=xTs[:, rt * P:(rt + 1) * P],
                                             rhs=eTh[:, c0:c0 + 512],
                                             start=True, stop=False)
                        for q in range(4):
                            c0 = g * GW + q * 512
                            nc.tensor.matmul(ps[:, q * 512:(q + 1) * 512],
                                             lhsT=ones_row[:],
                                             rhs=ny2h[:, c0:c0 + 512],
                                             start=False, stop=True)
                        nc.scalar.copy(S_sb[:, g * GW:(g + 1) * GW], ps[:])

                    flush_eg()
                    # fold cascade: tensor_tensor max, bf16 2x mode.
                    # fold1 pairs columns within each PSUM group so each
                    # piece starts right after that group's copy lands.
                    fold1 = pfold.tile([P, 5120], BF16, tag="fold1")
                    for g in range(GROUPS):
                        nc.vector.tensor_tensor(
                            out=fold1[:, g * 1024:(g + 1) * 1024],
                            in0=S_sb[:, g * GW:g * GW + 1024],
                            in1=S_sb[:, g * GW + 1024:(g + 1) * GW],
                            op=mybir.AluOpType.max)
                    fold2 = pfold.tile([P, 2560], BF16, tag="fold2")
                    nc.vector.tensor_tensor(out=fold2[:], in0=fold1[:, 0:2560],
                                            in1=fold1[:, 2560:5120],
                                            op=mybir.AluOpType.max)
                    fold3 = pfold.tile([P, 1280], BF16, tag="fold3")
                    nc.vector.tensor_tensor(out=fold3[:], in0=fold2[:, 0:1280],
                                            in1=fold2[:, 1280:2560],
                                            op=mybir.AluOpType.max)

                    # candidates: 2 blocks of 640 -> 16; merge for top-9
                    cand = psmall.tile([P, 16], BF16, tag="cand")
                    nc.vector.max(out=cand[:, 0:8], in_=fold3[:, 0:640])
                    nc.vector.max(out=cand[:, 8:16], in_=fold3[:, 640:1280])
                    m1 = psmall.tile([P, 8], BF16, tag="m1")
                    nc.vector.max(out=m1[:], in_=cand[:])
                    candz = psmall.tile([P, 16], BF16, tag="candz")
                    nc.vector.match_replace(out=candz[:], in_to_replace=m1[:],
                                            in_values=cand[:], imm_value=NEG_BIG)
                    m2 = psmall.tile([P, 8], BF16, tag="m2")
                    nc.vector.max(out=m2[:], in_=candz[:])
                    nv = psmall.tile([P, 8], BF16, tag="nv")
                    nc.vector.tensor_copy(nv[:, 0:7], m1[:, 1:8])
                    nc.vector.tensor_copy(nv[:, 7:8], m2[:, 0:1])

                    # d2_k = S_self - S_k
                    nc.vector.tensor_tensor(out=d2all[rt][:, iu * 8:(iu + 1) * 8],
                                            in0=m1[:, 0:1].to_broadcast([P, 8]),
                                            in1=nv[:],
                                            op=mybir.AluOpType.subtract)
                    # positions of nv in the full row (exact, one pass)
                    idx8 = pidx.tile([P, 8], U32, tag="idx8")
                    nc.vector.max_index(idx8[:], nv[:], S_sb[:])

                    # gather the 8 neighbor embedding rows (one DMA per k)
                    erows = psmall.tile([P, 8 * D], BF16, tag="erows")
                    for kk in range(8):
                        nc.gpsimd.indirect_dma_start(
                            out=erows[:, kk * D:(kk + 1) * D], out_offset=None,
                            in_=emb[h][:, :],
                            in_offset=bass.IndirectOffsetOnAxis(
                                ap=idx8[:, kk:kk + 1], axis=0))

                    # adjacency label bit: n_k in this edge-side's one-list
                    n16 = psmall.tile([P, 8], U16, tag="n16")
                    nc.vector.tensor_copy(n16[:], idx8[:])
                    aeq = psmall.tile([P, 8 * ALIST], F32, tag="aeq")
                    aeq3 = aeq[:].rearrange("p (o d) -> p o d", o=8)
                    n3 = n16[:].rearrange("p (o d) -> p o d", d=1).to_broadcast(
                        [P, 8, ALIST])
                    al3 = alist_t[(bu, rt)][:].rearrange(
                        "p (o d) -> p o d", o=1).to_broadcast([P, 8, ALIST])
                    nc.vector.tensor_tensor(out=aeq3, in0=n3, in1=al3,
                                            op=mybir.AluOpType.is_equal)
                    nc.vector.tensor_reduce(a8all[rt][:, iu * 8:(iu + 1) * 8],
                                            aeq3, axis=mybir.AxisListType.X,
                                            op=mybir.AluOpType.add)

                    # EG_k = e_s . g_b
                    gtile = psmall.tile([P, D], BF16, tag="gtile")
                    nc.sync.dma_start(out=gtile[:],
                                      in_=grows[h, bu, rt * P:(rt + 1) * P, :])
                    prod = psmall.tile([P, 8 * D], F32, tag="prod")
                    e3 = erows[:].rearrange("p (o d) -> p o d", o=8)
                    g3 = gtile[:].rearrange("p (o d) -> p o d", o=1).to_broadcast(
                        [P, 8, D])
                    p3 = prod[:].rearrange("p (o d) -> p o d", o=8)
                    nc.gpsimd.tensor_tensor(out=p3, in0=e3, in1=g3,
                                            op=mybir.AluOpType.mult)
                    pending_eg.append((p3, egall[rt][:, iu * 8:(iu + 1) * 8]))


        flush_eg()
        # ---- batched epilogue (per row tile, all (h,bu) at once) ----
        dist, wts = {}, {}
        for rt in range(RT):
            dist[rt] = pepi.tile([P, NU * 8], F32, tag=f"dist{rt}", name=f"dist{rt}")
            nc.scalar.sqrt(dist[rt][:], d2all[rt][:])
        for rt in range(RT):
            wts[rt] = pepi.tile([P, NU * 8], F32, tag=f"wts{rt}", name=f"wts{rt}")
            nc.scalar.activation(wts[rt][:], dist[rt][:],
                                 mybir.ActivationFunctionType.Exp,
                                 bias=1.0, scale=-1.0)
        acc = {}
        for rt in range(RT):
            # t1 = 2u*a8 - eg ; lg = t1 + xg - u
            t1 = pepi.tile([P, NU * 8], F32, tag="t1")
            nc.vector.scalar_tensor_tensor(out=t1[:], in0=a8all[rt][:],
                                           scalar=2.0 * u, in1=egall[rt][:],
                                           op0=mybir.AluOpType.mult,
                                           op1=mybir.AluOpType.subtract)
            xgb = xgall[rt][:].rearrange("p (o d) -> p o d", d=1).to_broadcast(
                [P, NU, 8])
            t13 = t1[:].rearrange("p (o d) -> p o d", o=NU)
            t2 = pepi.tile([P, NU * 8], F32, tag="t2")
            t23 = t2[:].rearrange("p (o d) -> p o d", o=NU)
            nc.vector.tensor_tensor(out=t23, in0=t13, in1=xgb,
                                    op=mybir.AluOpType.add)
            lg = pepi.tile([P, NU * 8], F32, tag="lg")
            nc.vector.tensor_scalar_add(lg[:], t2[:], -u)
            wl = pepi.tile([P, NU * 8], F32, tag="wl")
            nc.vector.tensor_tensor(out=wl[:], in0=wts[rt][:], in1=lg[:],
                                    op=mybir.AluOpType.mult)
            w3 = wts[rt][:].rearrange("p (o d) -> p o d", o=NU)
            wl3 = wl[:].rearrange("p (o d) -> p o d", o=NU)
            sw = pepi.tile([P, NU], F32, tag="sw")
            nc.vector.tensor_reduce(sw[:], w3, axis=mybir.AxisListType.X,
                                    op=mybir.AluOpType.add)
            swl = pepi.tile([P, NU], F32, tag="swl")
            nc.vector.tensor_reduce(swl[:], wl3, axis=mybir.AxisListType.X,
                                    op=mybir.AluOpType.add)
            # combine the two builds of each head: u = h*2 + bu
            sw3 = sw[:].rearrange("p (a b) -> p a b", a=H)
            swl3 = swl[:].rearrange("p (a b) -> p a b", a=H)
            swh = pepi.tile([P, H], F32, tag="swh")
            nc.vector.tensor_reduce(swh[:], sw3, axis=mybir.AxisListType.X,
                                    op=mybir.AluOpType.add)
            swlh = pepi.tile([P, H], F32, tag="swlh")
            nc.vector.tensor_reduce(swlh[:], swl3, axis=mybir.AxisListType.X,
                                    op=mybir.AluOpType.add)
            den = pepi.tile([P, H], F32, tag="den")
            nc.vector.tensor_scalar_add(den[:], swh[:], float(NSENT))
            rec = pepi.tile([P, H], F32, tag="rec")
            nc.vector.reciprocal(rec[:], den[:])
            smin = pepi.tile([P, H], F32, tag="smin")
            nc.vector.tensor_tensor(out=smin[:], in0=swlh[:], in1=rec[:],
                                    op=mybir.AluOpType.mult)
            acc[rt] = pepi.tile([P, 1], F32, tag=f"accr{rt}", name=f"accr{rt}")
            nc.vector.tensor_reduce(acc[rt][:], smin[:],
                                    axis=mybir.AxisListType.X,
                                    op=mybir.AluOpType.add)
        for rt in range(RT):
            sig = pepi.tile([P, 1], F32, tag=f"sig{rt}")
            nc.scalar.activation(sig[:], acc[rt][:],
                                 mybir.ActivationFunctionType.Sigmoid,
                                 scale=1.0 / H)
            nc.sync.dma_start(out=out_p[rt * P:(rt + 1) * P, :], in_=sig[:])

    nc.compile()
    return nc


def host_prep(embeds, field, uncertainty, adj, batch_edges):
    embeds = np.asarray(embeds, np.float32)
    field = np.asarray(field, np.float32)
    adj_u8 = (np.asarray(adj) != 0.0).astype(np.uint8)
    src = np.asarray(batch_edges[0]).astype(np.int64)
    dst = np.asarray(batch_edges[1]).astype(np.int64)

    eT_pad = np.zeros((H, D, NPAD), dtype=bf)
    eT_pad[:, :, :N] = embeds.transpose(0, 2, 1).astype(bf)
    y2 = np.sum(embeds * embeds, axis=2)                     # (H, N) f32
    ny2_pad = np.full((H, 1, NPAD), -30000.0, dtype=bf)
    ny2_pad[:, 0, :N] = (-y2).astype(bf)
    emb_rows = [np.ascontiguousarray(embeds[hh]).astype(bf) for hh in range(H)]

    in_maps = []
    for m in range(NCORES):
        sl = slice(m * NB, (m + 1) * NB)
        s_sh, d_sh = src[sl], dst[sl]
        nodes = {0: s_sh, 1: d_sh}

        xT2 = np.empty((H, 2, D, NB), dtype=bf)
        grows_np = np.empty((H, 2, NB, D), dtype=bf)
        xg_np = np.empty((RT, P, NU), dtype=np.float32)
        for bu in range(2):
            xe = embeds[:, nodes[bu], :]                     # (H, NB, D)
            ge = field[:, nodes[1 - bu], :]                  # (H, NB, D)
            xT2[:, bu] = (2.0 * xe).transpose(0, 2, 1).astype(bf)
            grows_np[:, bu] = ge.astype(bf)
            xg_hb = np.sum(xe * ge, axis=2)                  # (H, NB)
            for hh in range(H):
                xg_np[:, :, hh * 2 + bu] = xg_hb[hh].reshape(RT, P)

        alist_np = np.full((2, RT, P, ALIST), 0xFFFF, dtype=np.uint16)
        for rt in range(RT):
            rsl = slice(rt * P, (rt + 1) * P)
            for pp in range(P):
                # build0 label: adj[s_k, dst_b] -> ones of column dst_p
                nz = np.nonzero(adj_u8[:, d_sh[rsl][pp]])[0][:ALIST]
                alist_np[0, rt, pp, :len(nz)] = nz
                # build1 label: adj[src_b, s_k] -> ones of row src_p
                nz = np.nonzero(adj_u8[s_sh[rsl][pp], :])[0][:ALIST]
                alist_np[1, rt, pp, :len(nz)] = nz
        im = {"eT": eT_pad, "negy2": ny2_pad, "xT2": xT2,
              "grows": grows_np, "xg": xg_np, "alist": alist_np}
        for hh in range(H):
            im[f"emb_{hh}"] = emb_rows[hh]
        in_maps.append(im)
    return in_maps


_CACHE = {}


def kernel(embeds, field, uncertainty, adj, batch_edges, _profile=None):
    """Full inputs in, full (4096,) f32 output. Runs on NeuronCores 0-7."""
    u = float(np.asarray(uncertainty).reshape(-1)[0])
    if ('nc', u) not in _CACHE:
        _CACHE[('nc', u)] = build_kernel(u)
    nc = _CACHE[('nc', u)]
    in_maps = host_prep(embeds, field, uncertainty, adj, batch_edges)
    res = run_bass_kernel_spmd(nc, in_maps, list(range(NCORES)),
                               trace=bool(_profile))
    if isinstance(_profile, dict):
        _profile['exec_time_ns'] = res.exec_time_ns
        _profile['res'] = res
    return np.concatenate([np.asarray(res.results[i]["out"], np.float32).reshape(-1)
                           for i in range(NCORES)])


# revision 14
# speedup vs baseline: 1.0404x; 1.0404x over previous
"""MAD predictor (retrieval_knn) — Trainium2 Bass/Tile kernel on 8 NeuronCores.

kernel(**inputs) takes the FULL inputs and returns the FULL (4096,) f32 output.
Sharding: batch edges split 512/core across 8 cores; embeds replicated (bf16).
Per-edge gathers of *inputs* (adjacency rows/cols, field rows, x.g dots) are
host-staged as part of sharding; everything depending on *computed* k-NN
samples runs on device.

Per core, per head h and build (src->dst, dst->src), per 128-edge row tile:
  S[b,n] = 2*x_b.e_n - |e_n|^2   PE matmul in 5 groups of 4x512 cols + K=1
                                 bias matmul (negy2 host-precomputed, cols
                                 padded to 10240 with -30000)
  ACT copies each [128,2048] PSUM group to SBUF bf16.
  DVE fold cascade (tensor_tensor max, 2x bf16): 10240->5120->2560->1280;
  block max8 + match_replace merge on the 1280-wide fold gives self (m1[0])
  and the 8 nearest neighbors nv; find_index8 on the 1280-wide fold + 3
  gather/compare resolve levels (gpsimd indirect_copy) recover column ids.
  One batched indirect DMA gathers the 8 neighbor embedding rows; EG_k =
  e_s.g_b via gpsimd bcast-multiply + DVE grouped reduce; adjacency bits via
  indirect DMA from host-staged adj rows/cols (uint8).
  d2_k = S_self - S_k; per-rt batched epilogue: dist=sqrt(d2), w=exp(1-dist),
  logit = xg - EG + u*(2*adj-1); softmin_h = sum w*logit/(8+sum w);
  out = sigmoid(mean_h softmin_h).
"""

import sys
from contextlib import ExitStack

for _p in ('/opt/trn_rl_repo', '/root/.axon_site/_ro/trn_rl_repo'):
    if _p not in sys.path:
        sys.path.append(_p)

import numpy as np
import ml_dtypes

import concourse.bass as bass
import concourse.bacc as bacc
import concourse.mybir as mybir
from concourse.tile import TileContext
from concourse.bass_utils import run_bass_kernel_spmd

BF16 = mybir.dt.bfloat16
F32 = mybir.dt.float32
U32 = mybir.dt.uint32
U16 = mybir.dt.uint16
U8 = mybir.dt.uint8
P = 128
NEG_BIG = -3.0e38
bf = ml_dtypes.bfloat16

# problem constants (hardcoded per contract)
H, N, D = 4, 10000, 128
B, NCORES = 4096, 8
NB = B // NCORES          # 512 edges per core
RT = NB // P              # 4 row-tiles of 128 edges
NSENT = 8
NPAD = 10240              # 20 chunks of 512 exactly
ALIST = 32                # max adjacency-ones per edge side (P(>32) ~ 0)
GROUPS, GW = 5, 2048      # 5 PSUM groups of 4x512
NU = 2 * H                # (h, bu) units per rt


def build_kernel(u):
    nc = bacc.Bacc("TRN2", target_bir_lowering=False, debug=False,
                   enable_asserts=True, num_devices=NCORES)

    eT = nc.declare_dram_parameter("eT", [H, D, NPAD], BF16, isOutput=False)
    negy2 = nc.declare_dram_parameter("negy2", [H, 1, NPAD], BF16, isOutput=False)
    xT2 = nc.declare_dram_parameter("xT2", [H, 2, D, NB], BF16, isOutput=False)
    grows = nc.declare_dram_parameter("grows", [H, 2, NB, D], BF16, isOutput=False)
    xg_d = nc.declare_dram_parameter("xg", [RT, P, NU], F32, isOutput=False)
    emb = {h: nc.declare_dram_parameter(f"emb_{h}", [N, D], BF16, isOutput=False)
           for h in range(H)}
    # per edge-side adjacency: the <=ALIST column ids n with adj-label 1
    alist_d = nc.declare_dram_parameter("alist", [2, RT, P, ALIST], U16,
                                        isOutput=False)
    out_p = nc.declare_dram_parameter("out", [NB, 1], F32, isOutput=True)

    with TileContext(nc) as tc, ExitStack() as ctx:
        pconst = ctx.enter_context(tc.tile_pool(name="const", bufs=1))
        peT = ctx.enter_context(tc.tile_pool(name="peT", bufs=2))
        pS = ctx.enter_context(tc.tile_pool(name="pS", bufs=3))
        pfold = ctx.enter_context(tc.tile_pool(name="pfold", bufs=2))
        psmall = ctx.enter_context(tc.tile_pool(name="small", bufs=3))
        pacc = ctx.enter_context(tc.tile_pool(name="acc", bufs=1))
        pepi = ctx.enter_context(tc.tile_pool(name="epi", bufs=1))
        pny2 = ctx.enter_context(tc.tile_pool(name="ny2", bufs=1))
        pidx = ctx.enter_context(tc.tile_pool(name="idx", bufs=8))
        ppsum = ctx.enter_context(tc.tile_pool(name="psum", bufs=2, space="PSUM"))

        ones_row = pconst.tile([1, P], BF16)
        nc.vector.memset(ones_row[:], 1.0)
        alist_t = {}
        for bu in range(2):
            for rt in range(RT):
                alist_t[(bu, rt)] = pconst.tile([P, ALIST], U16,
                                                tag=f"al{bu}_{rt}",
                                                name=f"al{bu}_{rt}")
                nc.sync.dma_start(out=alist_t[(bu, rt)][:], in_=alist_d[bu, rt])

        # batched-epilogue accumulators, one set per row tile (flat layout)
        d2all = {rt: pacc.tile([P, NU * 8], F32, tag=f"d2all{rt}",
                               name=f"d2all{rt}") for rt in range(RT)}
        egall = {rt: pacc.tile([P, NU * 8], F32, tag=f"egall{rt}",
                               name=f"egall{rt}") for rt in range(RT)}
        a8all = {rt: pacc.tile([P, NU * 8], F32, tag=f"a8all{rt}",
                               name=f"a8all{rt}") for rt in range(RT)}
        xgall = {rt: pacc.tile([P, NU], F32, tag=f"xgall{rt}",
                               name=f"xgall{rt}") for rt in range(RT)}
        for rt in range(RT):
            nc.sync.dma_start(out=xgall[rt][:], in_=xg_d[rt])

        # PE warmup: sustained matmuls un-throttle the HAM clock gate.
        wsrc = pconst.tile([P, 512], BF16)
        nc.vector.memset(wsrc[:], 0.001)
        wps = ppsum.tile([P, GW], F32, tag="psS")
        for i in range(16):
            nc.tensor.matmul(wps[:, :512], lhsT=wsrc[:, :P], rhs=wsrc[:],
                             start=True, stop=True)
        wout = pconst.tile([1, 1], F32)
        nc.vector.tensor_copy(wout[:], wps[:1, :1])

        pending_eg = []

        def flush_eg():
            while pending_eg:
                p3p, dstp = pending_eg.pop(0)
                nc.vector.tensor_reduce(dstp, p3p, axis=mybir.AxisListType.X,
                                        op=mybir.AluOpType.add)

        for h in range(H):
            eTh = peT.tile([D, NPAD], BF16, tag="eTh")
            nc.sync.dma_start(out=eTh[:], in_=eT[h])
            ny2h = pny2.tile([1, NPAD], BF16, tag="ny2h")
            nc.sync.dma_start(out=ny2h[:], in_=negy2[h])

            for bu in range(2):
                iu = h * 2 + bu
                xTs = psmall.tile([D, NB], BF16, tag="xTs")
                nc.sync.dma_start(out=xTs[:], in_=xT2[h, bu])

                for rt in range(RT):
                    S_sb = pS.tile([P, NPAD], BF16, tag="S_sb")
                    for g in range(GROUPS):
                        ps = ppsum.tile([P, GW], F32, tag="psS")
                        for q in range(4):
                            c0 = g * GW + q * 512
                            nc.tensor.matmul(ps[:, q * 512:(q + 1) * 512],
                                             lhsT=xTs[:, rt * P:(rt + 1) * P],
                                             rhs=eTh[:, c0:c0 + 512],
                                             start=True, stop=False)
                        for q in range(4):
                            c0 = g * GW + q * 512
                            nc.tensor.matmul(ps[:, q * 512:(q + 1) * 512],
                                             lhsT=ones_row[:],
                                             rhs=ny2h[:, c0:c0 + 512],
                                             start=False, stop=True)
                        nc.scalar.copy(S_sb[:, g * GW:(g + 1) * GW], ps[:])

                    flush_eg()
                    # fold cascade: tensor_tensor max, bf16 2x mode
                    fold1 = pfold.tile([P, 5120], BF16, tag="fold1")
                    nc.vector.tensor_tensor(out=fold1[:], in0=S_sb[:, 0:5120],
                                            in1=S_sb[:, 5120:10240],
                                            op=mybir.AluOpType.max)
                    fold2 = pfold.tile([P, 2560], BF16, tag="fold2")
                    nc.vector.tensor_tensor(out=fold2[:], in0=fold1[:, 0:2560],
                                            in1=fold1[:, 2560:5120],
                                            op=mybir.AluOpType.max)
                    fold3 = pfold.tile([P, 1280], BF16, tag="fold3")
                    nc.vector.tensor_tensor(out=fold3[:], in0=fold2[:, 0:1280],
                                            in1=fold2[:, 1280:2560],
                                            op=mybir.AluOpType.max)

                    # candidates: 2 blocks of 640 -> 16; merge for top-9
                    cand = psmall.tile([P, 16], BF16, tag="cand")
                    nc.vector.max(out=cand[:, 0:8], in_=fold3[:, 0:640])
                    nc.vector.max(out=cand[:, 8:16], in_=fold3[:, 640:1280])
                    m1 = psmall.tile([P, 8], BF16, tag="m1")
                    nc.vector.max(out=m1[:], in_=cand[:])
                    candz = psmall.tile([P, 16], BF16, tag="candz")
                    nc.vector.match_replace(out=candz[:], in_to_replace=m1[:],
                                            in_values=cand[:], imm_value=NEG_BIG)
                    m2 = psmall.tile([P, 8], BF16, tag="m2")
                    nc.vector.max(out=m2[:], in_=candz[:])
                    nv = psmall.tile([P, 8], BF16, tag="nv")
                    nc.vector.tensor_copy(nv[:, 0:7], m1[:, 1:8])
                    nc.vector.tensor_copy(nv[:, 7:8], m2[:, 0:1])

                    # d2_k = S_self - S_k
                    nc.vector.tensor_tensor(out=d2all[rt][:, iu * 8:(iu + 1) * 8],
                                            in0=m1[:, 0:1].to_broadcast([P, 8]),
                                            in1=nv[:],
                                            op=mybir.AluOpType.subtract)
                    # positions of nv in the full row (exact, one pass)
                    idx8 = pidx.tile([P, 8], U32, tag="idx8")
                    nc.vector.max_index(idx8[:], nv[:], S_sb[:])

                    # gather the 8 neighbor embedding rows (one DMA per k)
                    erows = psmall.tile([P, 8 * D], BF16, tag="erows")
                    for kk in range(8):
                        nc.gpsimd.indirect_dma_start(
                            out=erows[:, kk * D:(kk + 1) * D], out_offset=None,
                            in_=emb[h][:, :],
                            in_offset=bass.IndirectOffsetOnAxis(
                                ap=idx8[:, kk:kk + 1], axis=0))

                    # adjacency label bit: n_k in this edge-side's one-list
                    n16 = psmall.tile([P, 8], U16, tag="n16")
                    nc.vector.tensor_copy(n16[:], idx8[:])
                    aeq = psmall.tile([P, 8 * ALIST], F32, tag="aeq")
                    aeq3 = aeq[:].rearrange("p (o d) -> p o d", o=8)
                    n3 = n16[:].rearrange("p (o d) -> p o d", d=1).to_broadcast(
                        [P, 8, ALIST])
                    al3 = alist_t[(bu, rt)][:].rearrange(
                        "p (o d) -> p o d", o=1).to_broadcast([P, 8, ALIST])
                    nc.vector.tensor_tensor(out=aeq3, in0=n3, in1=al3,
                                            op=mybir.AluOpType.is_equal)
                    nc.vector.tensor_reduce(a8all[rt][:, iu * 8:(iu + 1) * 8],
                                            aeq3, axis=mybir.AxisListType.X,
                                            op=mybir.AluOpType.add)

                    # EG_k = e_s . g_b
                    gtile = psmall.tile([P, D], BF16, tag="gtile")
                    nc.sync.dma_start(out=gtile[:],
                                      in_=grows[h, bu, rt * P:(rt + 1) * P, :])
                    prod = psmall.tile([P, 8 * D], F32, tag="prod")
                    e3 = erows[:].rearrange("p (o d) -> p o d", o=8)
                    g3 = gtile[:].rearrange("p (o d) -> p o d", o=1).to_broadcast(
                        [P, 8, D])
                    p3 = prod[:].rearrange("p (o d) -> p o d", o=8)
                    nc.gpsimd.tensor_tensor(out=p3, in0=e3, in1=g3,
                                            op=mybir.AluOpType.mult)
                    pending_eg.append((p3, egall[rt][:, iu * 8:(iu + 1) * 8]))


        flush_eg()
        # ---- batched epilogue (per row tile, all (h,bu) at once) ----
        dist, wts = {}, {}
        for rt in range(RT):
            dist[rt] = pepi.tile([P, NU * 8], F32, tag=f"dist{rt}", name=f"dist{rt}")
            nc.scalar.sqrt(dist[rt][:], d2all[rt][:])
        for rt in range(RT):
            wts[rt] = pepi.tile([P, NU * 8], F32, tag=f"wts{rt}", name=f"wts{rt}")
            nc.scalar.activation(wts[rt][:], dist[rt][:],
                                 mybir.ActivationFunctionType.Exp,
                                 bias=1.0, scale=-1.0)
        acc = {}
        for rt in range(RT):
            # t1 = 2u*a8 - eg ; lg = t1 + xg - u
            t1 = pepi.tile([P, NU * 8], F32, tag="t1")
            nc.vector.scalar_tensor_tensor(out=t1[:], in0=a8all[rt][:],
                                           scalar=2.0 * u, in1=egall[rt][:],
                                           op0=mybir.AluOpType.mult,
                                           op1=mybir.AluOpType.subtract)
            xgb = xgall[rt][:].rearrange("p (o d) -> p o d", d=1).to_broadcast(
                [P, NU, 8])
            t13 = t1[:].rearrange("p (o d) -> p o d", o=NU)
            t2 = pepi.tile([P, NU * 8], F32, tag="t2")
            t23 = t2[:].rearrange("p (o d) -> p o d", o=NU)
            nc.vector.tensor_tensor(out=t23, in0=t13, in1=xgb,
                                    op=mybir.AluOpType.add)
            lg = pepi.tile([P, NU * 8], F32, tag="lg")
            nc.vector.tensor_scalar_add(lg[:], t2[:], -u)
            wl = pepi.tile([P, NU * 8], F32, tag="wl")
            nc.vector.tensor_tensor(out=wl[:], in0=wts[rt][:], in1=lg[:],
                                    op=mybir.AluOpType.mult)
            w3 = wts[rt][:].rearrange("p (o d) -> p o d", o=NU)
            wl3 = wl[:].rearrange("p (o d) -> p o d", o=NU)
            sw = pepi.tile([P, NU], F32, tag="sw")
            nc.vector.tensor_reduce(sw[:], w3, axis=mybir.AxisListType.X,
                                    op=mybir.AluOpType.add)
            swl = pepi.tile([P, NU], F32, tag="swl")
            nc.vector.tensor_reduce(swl[:], wl3, axis=mybir.AxisListType.X,
                                    op=mybir.AluOpType.add)
            # combine the two builds of each head: u = h*2 + bu
            sw3 = sw[:].rearrange("p (a b) -> p a b", a=H)
            swl3 = swl[:].rearrange("p (a b) -> p a b", a=H)
            swh = pepi.tile([P, H], F32, tag="swh")
            nc.vector.tensor_reduce(swh[:], sw3, axis=mybir.AxisListType.X,
                                    op=mybir.AluOpType.add)
            swlh = pepi.tile([P, H], F32, tag="swlh")
            nc.vector.tensor_reduce(swlh[:], swl3, axis=mybir.AxisListType.X,
                                    op=mybir.AluOpType.add)
            den = pepi.tile([P, H], F32, tag="den")
            nc.vector.tensor_scalar_add(den[:], swh[:], float(NSENT))
            rec = pepi.tile([P, H], F32, tag="rec")
            nc.vector.reciprocal(rec[:], den[:])
            smin = pepi.tile([P, H], F32, tag="smin")
            nc.vector.tensor_tensor(out=smin[:], in0=swlh[:], in1=rec[:],
                                    op=mybir.AluOpType.mult)
            acc[rt] = pepi.tile([P, 1], F32, tag=f"accr{rt}", name=f"accr{rt}")
            nc.vector.tensor_reduce(acc[rt][:], smin[:],
                                    axis=mybir.AxisListType.X,
                                    op=mybir.AluOpType.add)
        for rt in range(RT):
            sig = pepi.tile([P, 1], F32, tag=f"sig{rt}")
            nc.scalar.activation(sig[:], acc[rt][:],
                                 mybir.ActivationFunctionType.Sigmoid,
                                 scale=1.0 / H)
            nc.sync.dma_start(out=out_p[rt * P:(rt + 1) * P, :], in_=sig[:])

    nc.compile()
    return nc


def host_prep(embeds, field, uncertainty, adj, batch_edges):
    embeds = np.asarray(embeds, np.float32)
    field = np.asarray(field, np.float32)
    adj_u8 = (np.asarray(adj) != 0.0).astype(np.uint8)
    src = np.asarray(batch_edges[0]).astype(np.int64)
    dst = np.asarray(batch_edges[1]).astype(np.int64)

    eT_pad = np.zeros((H, D, NPAD), dtype=bf)
    eT_pad[:, :, :N] = embeds.transpose(0, 2, 1).astype(bf)
    y2 = np.sum(embeds * embeds, axis=2)                     # (H, N) f32
    ny2_pad = np.full((H, 1, NPAD), -30000.0, dtype=bf)
    ny2_pad[:, 0, :N] = (-y2).astype(bf)
    emb_rows = [np.ascontiguousarray(embeds[hh]).astype(bf) for hh in range(H)]

    in_maps = []
    for m in range(NCORES):
        sl = slice(m * NB, (m + 1) * NB)
        s_sh, d_sh = src[sl], dst[sl]
        nodes = {0: s_sh, 1: d_sh}

        xT2 = np.empty((H, 2, D, NB), dtype=bf)
        grows_np = np.empty((H, 2, NB, D), dtype=bf)
        xg_np = np.empty((RT, P, NU), dtype=np.float32)
        for bu in range(2):
            xe = embeds[:, nodes[bu], :]                     # (H, NB, D)
            ge = field[:, nodes[1 - bu], :]                  # (H, NB, D)
            xT2[:, bu] = (2.0 * xe).transpose(0, 2, 1).astype(bf)
            grows_np[:, bu] = ge.astype(bf)
            xg_hb = np.sum(xe * ge, axis=2)                  # (H, NB)
            for hh in range(H):
                xg_np[:, :, hh * 2 + bu] = xg_hb[hh].reshape(RT, P)

        alist_np = np.full((2, RT, P, ALIST), 0xFFFF, dtype=np.uint16)
        for rt in range(RT):
            rsl = slice(rt * P, (rt + 1) * P)
            for pp in range(P):
                # build0 label: adj[s_k, dst_b] -> ones of column dst_p
                nz = np.nonzero(adj_u8[:, d_sh[rsl][pp]])[0][:ALIST]
                alist_np[0, rt, pp, :len(nz)] = nz
                # build1 label: adj[src_b, s_k] -> ones of row src_p
                nz = np.nonzero(adj_u8[s_sh[rsl][pp], :])[0][:ALIST]
                alist_np[1, rt, pp, :len(nz)] = nz
        im = {"eT": eT_pad, "negy2": ny2_pad, "xT2": xT2,
              "grows": grows_np, "xg": xg_np, "alist": alist_np}
        for hh in range(H):
            im[f"emb_{hh}"] = emb_rows[hh]
        in_maps.append(im)
    return in_maps


_CACHE = {}


def kernel(embeds, field, uncertainty, adj, batch_edges, _profile=None):
    """Full inputs in, full (4096,) f32 output. Runs on NeuronCores 0-7."""
    u = float(np.asarray(uncertainty).reshape(-1)[0])
    if ('nc', u) not in _CACHE:
        _CACHE[('nc', u)] = build_kernel(u)
    nc = _CACHE[('nc', u)]
    in_maps = host_prep(embeds, field, uncertainty, adj, batch_edges)
    res = run_bass_kernel_spmd(nc, in_maps, list(range(NCORES)),
                               trace=bool(_profile))
    if isinstance(_profile, dict):
        _profile['exec_time_ns'] = res.exec_time_ns
        _profile['res'] = res
    return np.concatenate([np.asarray(res.results[i]["out"], np.float32).reshape(-1)
                           for i in range(NCORES)])
